# revision 1
# baseline (speedup 1.0000x reference)
"""Fused self-attention + residual + LayerNorm kernel for Trainium2.

Reference computation (per batch b of 16):
    S    = x @ x.T                  [2048, 2048]
    A    = softmax(S, axis=-1)
    out  = A @ x                    [2048, 128]
    y    = out + x
    res  = LayerNorm(y) * gamma + beta

Sharding: data-parallel over batch, 2 batches per core on 8 NeuronCores (SPMD,
no collectives).

Algorithm notes (per core / per batch):
  * Softmax stabilization without a max pass: with c_q = ||x_q||^2 and
    cbar = max_k c_k, Cauchy-Schwarz gives S[q,k] <= (c_q + c_k)/2, so
    P[q,k] = exp(S[q,k] - (c_q + cbar)/2) <= 1 never overflows.  P's row sums
    (the softmax denominators, up to the same per-row shift) come for free
    from the ACT engine's accum_out.
  * The AV matmul needs attention weights with k on the partition axis.  S is
    symmetric, so E = exp(S - c_q/2 - c_k/2) (= P * g_k with
    g_k = exp((cbar - c_k)/2)) is symmetric and its stored q-major tiles can
    be consumed directly as k-major operand slices -- no transposes anywhere.
    The AV contraction uses scaled values Vt[k,:] = t_k * x[k,:]
    (t_k = 1/g_k), which makes num[q,:] = sum_k P[q,k] x[k,:] exactly,
    consistent with the accumulated denominators.
  * QK^T and AV run in bf16 (f32 PSUM accumulation); rsqrt for LayerNorm is
    exp(-0.5*ln(var+eps)) so the whole kernel stays on one ACT table set.
  * The two batches are software-pipelined: batch 1's main loop overlaps
    batch 0's output stage, and each engine's issue order is time-monotone.
"""

import sys

import numpy as np

sys.path.insert(0, "/opt/trn_rl_repo")

B, T, D = 16, 2048, 128
N_CORES = 8
NB = B // N_CORES          # batches per core
NT = T // 128              # 128-row tiles per batch
EPS = 1e-5

_CACHE = {}


def _build():
    from contextlib import ExitStack

    import concourse.bacc as bacc
    import concourse.bass as bass  # noqa: F401
    import concourse.tile as tile
    from concourse import mybir

    f32 = mybir.dt.float32
    bf = mybir.dt.bfloat16
    AF = mybir.ActivationFunctionType
    ALU = mybir.AluOpType
    AX = mybir.AxisListType

    nc = bacc.Bacc()

    x_d = nc.dram_tensor("x", [NB, T, D], f32, kind="ExternalInput")
    xT_d = nc.dram_tensor("xT", [NB, D, T], bf, kind="ExternalInput")
    g_d = nc.dram_tensor("gamma", [D], f32, kind="ExternalInput")
    b_d = nc.dram_tensor("beta", [D], f32, kind="ExternalInput")
    o_d = nc.dram_tensor("out", [NB, T, D], f32, kind="ExternalOutput")
    g_scr = nc.dram_tensor("gscratch", [NB, T], bf, kind="Internal")

    ctx = ExitStack()
    with tile.TileContext(nc) as tc, ctx:
        big = ctx.enter_context(tc.tile_pool(name="big", bufs=2))
        epool = ctx.enter_context(tc.tile_pool(name="epool", bufs=8))
        stats = ctx.enter_context(tc.tile_pool(name="stats", bufs=2))
        consts = ctx.enter_context(tc.tile_pool(name="consts", bufs=1))
        spool = ctx.enter_context(tc.tile_pool(name="spool", bufs=2, space="PSUM"))
        npool = ctx.enter_context(tc.tile_pool(name="npool", bufs=1, space="PSUM"))

        zero_t = consts.tile([128, 1], f32, tag="zero", name="zero")
        nc.vector.memset(zero_t, 0.0)
        ones_c = consts.tile([128, 1], f32, tag="ones_c", name="ones_c")
        nc.vector.memset(ones_c, 1.0)
        ones_r = consts.tile([1, 128], f32, tag="ones_r", name="ones_r")
        nc.vector.memset(ones_r, 1.0)

        def emit_loads(b, st):
            st["xT"] = big.tile([128, T], bf, tag="xT", name="xT")
            st["x"] = big.tile([128, NT, D], f32, tag="x", name="x")
            xv = x_d[b].rearrange("(t p) d -> p t d", p=128)
            for sx in range(4):
                nc.sync.dma_start(
                    out=st["x"][:, sx * 4 : (sx + 1) * 4, :],
                    in_=xv[:, sx * 4 : (sx + 1) * 4, :],
                )

        def emit_loads_xT(b, st):
            for sx in range(2):
                nc.sync.dma_start(
                    out=st["xT"][:, sx * 1024 : (sx + 1) * 1024],
                    in_=xT_d[b, :, sx * 1024 : (sx + 1) * 1024],
                )

        def emit_stats(b, st):
            x_sb = st["x"]
            C = stats.tile([128, NT], f32, tag="C", name="C")
            sqb = big.tile([128, NT, D], f32, tag="sqb", name="sqb")
            for t in range(NT):
                nc.vector.scalar_tensor_tensor(
                    out=sqb[:, t, :],
                    in0=x_sb[:, t, :],
                    scalar=1.0,
                    in1=x_sb[:, t, :],
                    op0=ALU.mult,
                    op1=ALU.mult,
                    accum_out=C[:, t : t + 1],
                )
            # upper bound on max(c) without any cross-partition reduction:
            # cbar = 6*ln(sum_k exp(c_k/6)) in [max c, max c + 6 ln 2048]
            # (6 keeps the sum under ACT-ln's 2^64 input limit).
            # The cross-partition sum and the broadcast back are both K=1/M=1
            # matmuls on the PE -- no DRAM bounce, ~2us total latency.
            EC = stats.tile([128, NT], f32, tag="EC", name="EC")
            nc.scalar.activation(out=EC, in_=C, func=AF.Exp, bias=zero_t, scale=1.0 / 6.0)
            ec1 = stats.tile([128, 1], f32, tag="ec1", name="ec1")
            nc.vector.tensor_reduce(out=ec1, in_=EC, axis=AX.X, op=ALU.add)
            # cross-partition sum and partition-broadcast as K=1/M=1 matmuls,
            # ln via float-bits (Schraudolph): float(bits(x)) ~
            # (log2(x)+126.94)*2^23.  cbar only needs to stay a near-tight
            # upper bound of max(c); the shift cancels exactly regardless.
            s1 = spool.tile([1, 1], f32, tag="S", name="s1")
            nc.tensor.matmul(out=s1, lhsT=ec1, rhs=ones_c, start=True, stop=True)
            LL = stats.tile([1, 1], f32, tag="LL", name="LL")
            nc.vector.tensor_copy(out=LL, in_=s1.bitcast(mybir.dt.int32))
            s2 = spool.tile([128, 1], f32, tag="S", name="s2")
            nc.tensor.matmul(out=s2, lhsT=ones_r, rhs=LL, start=True, stop=True)
            cmb = stats.tile([128, 1], f32, tag="cmb", name="cmb")
            LN2_6 = 6.0 * 0.6931471805599453
            nc.vector.tensor_scalar(
                out=cmb, in0=s2,
                scalar1=LN2_6 / 8388608.0, scalar2=-126.9412 * LN2_6,
                op0=ALU.mult, op1=ALU.add,
            )
            cmh = stats.tile([128, 1], f32, tag="cmh", name="cmh")
            cmhn = stats.tile([128, 1], f32, tag="cmhn", name="cmhn")
            nc.vector.tensor_scalar_mul(out=cmh, in0=cmb, scalar1=0.5)
            nc.vector.tensor_scalar_mul(out=cmhn, in0=cmb, scalar1=-0.5)
            # bias_all[:, t] = -(c + cbar)/2
            bias_all = stats.tile([128, NT], f32, tag="bias", name="bias")
            nc.vector.tensor_scalar(
                out=bias_all,
                in0=C,
                scalar1=cmb,
                scalar2=-0.5,
                op0=ALU.add,
                op1=ALU.mult,
            )
            st["bias"] = bias_all
            # g = exp((cbar - c)/2), t = 1/g; scaled values Vt = t * x (bf16)
            Gall = stats.tile([128, NT], bf, tag="Gall", name="Gall")
            nc.scalar.activation(out=Gall, in_=C, func=AF.Exp, bias=cmh, scale=-0.5)
            Tall = stats.tile([128, NT], f32, tag="Tall", name="Tall")
            nc.scalar.activation(out=Tall, in_=C, func=AF.Exp, bias=cmhn, scale=0.5)
            Vt = big.tile([128, NT, D], bf, tag="Vt", name="Vt")
            for t in range(NT):
                nc.vector.tensor_scalar_mul(
                    out=Vt[:, t, :], in0=x_sb[:, t, :], scalar1=Tall[:, t : t + 1]
                )
            st["Vt"] = Vt
            # broadcast g along partitions: Gb[p, k] = g_k (DRAM bounce)
            nc.sync.dma_start(
                out=g_scr[b].rearrange("(t p) -> p t", p=128), in_=Gall
            )
            Gb = big.tile([128, T], bf, tag="Gb", name="Gb")
            nc.sync.dma_start(out=Gb, in_=g_scr[b].partition_broadcast(128))
            st["Gb"] = Gb
            st["Den"] = stats.tile([128, NT, 2], f32, tag="Den", name="Den")

        def emit_main_step(b, st, j):
            if j == 0:
                st["num"] = npool.tile([128, T], f32, tag="num", name="num")
            E_j = epool.tile([128, T], bf, tag="E", name="E")
            xT_sb = st["xT"]
            for h in range(2):
                S = spool.tile([128, 1024], f32, tag="S", name="S")
                for q in range(2):
                    n0 = h * 1024 + q * 512
                    nc.tensor.matmul(
                        out=S[:, q * 512 : (q + 1) * 512],
                        lhsT=xT_sb[:, j * 128 : (j + 1) * 128],
                        rhs=xT_sb[:, n0 : n0 + 512],
                        start=True,
                        stop=True,
                    )
                nc.scalar.activation(
                    out=E_j[:, h * 1024 : (h + 1) * 1024],
                    in_=S,
                    func=AF.Exp,
                    bias=st["bias"][:, j : j + 1],
                    scale=1.0,
                    accum_out=st["Den"][:, j, h : h + 1],
                )
                eng = nc.vector if j % 2 == 0 else nc.gpsimd
                eng.tensor_mul(
                    out=E_j[:, h * 1024 : (h + 1) * 1024],
                    in0=E_j[:, h * 1024 : (h + 1) * 1024],
                    in1=st["Gb"][:, h * 1024 : (h + 1) * 1024],
                )
            for jj in range(NT):
                # 4 output slices share a 2KB PSUM bank = one zero region:
                # only the bank's first MM sets start, only its last sets stop
                nc.tensor.matmul(
                    out=st["num"][:, jj * 128 : (jj + 1) * 128],
                    lhsT=E_j[:, jj * 128 : (jj + 1) * 128],
                    rhs=st["Vt"][:, j, :],
                    start=(j == 0 and jj % 4 == 0),
                    stop=(j == NT - 1 and jj % 4 == 3),
                )

        def emit_den(b, st):
            den = stats.tile([128, NT], f32, tag="den", name="den")
            nc.vector.tensor_reduce(out=den, in_=st["Den"], axis=AX.X, op=ALU.add)
            R = stats.tile([128, NT], f32, tag="R", name="R")
            nc.vector.reciprocal(out=R, in_=den)
            st["R"] = R

        def emit_drain(b, st, copy_psum=True, half=None):
            # drain AV results out of PSUM so the next batch can reuse it
            # (skipped for the last batch -- nothing needs the banks).
            # Staggered: half 0 at the phase boundary, half 1 a few iterations
            # later, so the copies don't starve the E-mul stream on DVE.
            if copy_psum:
                if half != 1:
                    st["numS"] = big.tile([128, T], f32, tag="numS", name="numS")
                for h in ([0, 1] if half is None else [half]):
                    nc.vector.tensor_copy(
                        out=st["numS"][:, h * 1024 : (h + 1) * 1024],
                        in_=st["num"][:, h * 1024 : (h + 1) * 1024],
                    )
            else:
                st["numS"] = st["num"]
            if half != 1:
                st["Y"] = big.tile([128, NT, D], f32, tag="Y", name="Y")
                st["MV"] = stats.tile([128, NT, 2], f32, tag="MV", name="MV")
                st["Yout"] = big.tile([128, NT, D], f32, tag="Yout", name="Yout")

        def emit_outA(b, st, jj, act_stats=False):
            # y = num/den + x.  LN stats either via DVE bn_stats (b0: DVE has
            # slack mid-phase, ACT is the bottleneck) or via accum_out +
            # ACT Square (b1 tail: ACT is idle, DVE is the critical path).
            nc.vector.scalar_tensor_tensor(
                out=st["Y"][:, jj, :],
                in0=st["numS"][:, jj * 128 : (jj + 1) * 128],
                scalar=st["R"][:, jj : jj + 1],
                in1=st["x"][:, jj, :],
                op0=ALU.mult,
                op1=ALU.add,
                accum_out=st["Sy"][:, jj : jj + 1] if act_stats else None,
            )
            if act_stats:
                nc.vector.scalar_tensor_tensor(
                    out=st["ysqb"][:, jj, :],
                    in0=st["Y"][:, jj, :],
                    scalar=1.0,
                    in1=st["Y"][:, jj, :],
                    op0=ALU.mult,
                    op1=ALU.mult,
                    accum_out=st["Sy2"][:, jj : jj + 1],
                )
            else:
                bns = stats.tile([128, 6], f32, tag="bns", name="bns")
                nc.vector.bn_stats(out=bns, in_=st["Y"][:, jj, :])
                nc.vector.bn_aggr(out=st["MV"][:, jj, :], in_=bns)

        def emit_lnr(b, st, act_stats=False, lo=0, hi=NT):
            cs = slice(lo, hi)
            if act_stats:
                if "mu" not in st:
                    st["mu"] = stats.tile([128, NT], f32, tag="mu", name="mu")
                    st["vart"] = stats.tile([128, NT], f32, tag="vart", name="vart")
                    st["rstd"] = stats.tile([128, NT], f32, tag="rstd", name="rstd")
                    st["lnv"] = stats.tile([128, NT], f32, tag="lnv", name="lnv")
                # mu = Sy/128, var = Sy2/128 - mu^2
                nc.vector.tensor_scalar_mul(
                    out=st["mu"][:, cs], in0=st["Sy"][:, cs], scalar1=1.0 / D
                )
                musq = stats.tile([128, NT], f32, tag="musq", name="musq")
                nc.vector.scalar_tensor_tensor(
                    out=musq[:, cs],
                    in0=st["mu"][:, cs],
                    scalar=1.0,
                    in1=st["mu"][:, cs],
                    op0=ALU.mult,
                    op1=ALU.mult,
                )
                nc.vector.scalar_tensor_tensor(
                    out=st["vart"][:, cs],
                    in0=st["Sy2"][:, cs],
                    scalar=1.0 / D,
                    in1=musq[:, cs],
                    op0=ALU.mult,
                    op1=ALU.subtract,
                )
                var_in = st["vart"][:, cs]
            else:
                if "rstd" not in st:
                    st["rstd"] = stats.tile([128, NT], f32, tag="rstd", name="rstd")
                var_in = st["MV"][:, cs, 1]
            # rstd = 1/sqrt(var+eps) via the fast-inverse-sqrt bit trick plus
            # two Newton steps (~4e-6 rel err) -- keeps the ACT engine on the
            # exp table set for the whole kernel (table swaps cost 1.3us each)
            ve = stats.tile([128, NT], f32, tag="ve", name="ve")
            nc.vector.tensor_scalar_add(out=ve[:, cs], in0=var_in, scalar1=EPS)
            wf = stats.tile([128, NT], f32, tag="wf", name="wf")
            nc.vector.tensor_copy(out=wf[:, cs], in_=ve[:, cs].bitcast(mybir.dt.int32))
            nc.vector.tensor_scalar(
                out=wf[:, cs], in0=wf[:, cs],
                scalar1=-0.5, scalar2=1597463007.0,
                op0=ALU.mult, op1=ALU.add,
            )
            wi = stats.tile([128, NT], mybir.dt.int32, tag="wi", name="wi")
            nc.vector.tensor_copy(out=wi[:, cs], in_=wf[:, cs])
            y = stats.tile([128, NT], f32, tag="y0", name="y0")
            nc.vector.tensor_copy(out=y[:, cs], in_=wi[:, cs].bitcast(f32))
            t1 = stats.tile([128, NT], f32, tag="t1", name="t1")
            for _ in range(2):
                nc.vector.tensor_mul(out=t1[:, cs], in0=ve[:, cs], in1=y[:, cs])
                nc.vector.tensor_mul(out=t1[:, cs], in0=t1[:, cs], in1=y[:, cs])
                nc.vector.tensor_scalar(
                    out=t1[:, cs], in0=t1[:, cs],
                    scalar1=-0.5, scalar2=1.5, op0=ALU.mult, op1=ALU.add,
                )
                nc.vector.tensor_mul(out=y[:, cs], in0=y[:, cs], in1=t1[:, cs])
            nc.vector.tensor_copy(out=st["rstd"][:, cs], in_=y[:, cs])

        def emit_outB(b, st, jj, act_stats=False):
            mu_s = st["mu"][:, jj : jj + 1] if act_stats else st["MV"][:, jj, 0:1]
            z = stats.tile([128, D], f32, tag="z", name="z")
            nc.vector.tensor_scalar(
                out=z,
                in0=st["Y"][:, jj, :],
                scalar1=mu_s,
                scalar2=st["rstd"][:, jj : jj + 1],
                op0=ALU.subtract,
                op1=ALU.mult,
            )
            z2 = stats.tile([128, D], f32, tag="z2", name="z2")
            nc.gpsimd.tensor_mul(out=z2, in0=z, in1=gb)
            nc.gpsimd.tensor_add(out=st["Yout"][:, jj, :], in0=z2, in1=bb)

        def emit_outdma(b, st, half=None, quarter=None):
            ov = o_d[b].rearrange("(t p) d -> p t d", p=128)
            if quarter is not None:
                q4 = slice(quarter * 4, (quarter + 1) * 4)
                nc.sync.dma_start(out=ov[:, q4, :], in_=st["Yout"][:, q4, :])
            elif half is None:
                nc.sync.dma_start(out=ov, in_=st["Yout"])
            else:
                h8 = slice(half * 8, (half + 1) * 8)
                nc.sync.dma_start(out=ov[:, h8, :], in_=st["Yout"][:, h8, :])

        # ---- software-pipelined schedule over the two batches ---------------
        A, Bst = {}, {}
        emit_loads(0, A)
        emit_stats(0, A)
        emit_loads_xT(0, A)
        emit_loads(1, Bst)
        emit_loads_xT(1, Bst)
        gb = consts.tile([128, D], f32, tag="gb", name="gb")
        bb = consts.tile([128, D], f32, tag="bb", name="bb")
        for j in range(NT):
            emit_main_step(0, A, j)
            if j == 3:
                emit_stats(1, Bst)
            if j == 5:
                nc.sync.dma_start(out=gb, in_=g_d[:].partition_broadcast(128))
                nc.sync.dma_start(out=bb, in_=b_d[:].partition_broadcast(128))
        emit_den(0, A)
        emit_drain(0, A, half=0)
        # phase 1: batch 1's main loop with batch 0's whole output stage
        # threaded through it (outA x2 in early iters, lnr at 8, outB x2 late)
        for j in range(NT):
            emit_main_step(1, Bst, j)
            if j == 2:
                emit_drain(0, A, half=1)
            if j < 8:
                emit_outA(0, A, 2 * j)
                emit_outA(0, A, 2 * j + 1)
            else:
                if j == 8:
                    emit_lnr(0, A)
                emit_outB(0, A, 2 * (j - 8))
                emit_outB(0, A, 2 * (j - 8) + 1)
                if j == 12:
                    emit_outdma(0, A, half=0)
        emit_outdma(0, A, half=1)
        emit_den(1, Bst)
        emit_drain(1, Bst, copy_psum=False)
        Bst["Sy"] = stats.tile([128, NT], f32, tag="Sy", name="Sy")
        Bst["Sy2"] = stats.tile([128, NT], f32, tag="Sy2", name="Sy2")
        Bst["ysqb"] = big.tile([128, NT, D], f32, tag="ysqb", name="ysqb", bufs=1)
        # half-split tail: LN stats for tiles 0-7 finish while 8-15 are still
        # accumulating, so normalize+store of the first half overlaps the rest
        for jj in range(8):
            emit_outA(1, Bst, jj, act_stats=True)
        emit_lnr(1, Bst, act_stats=True, lo=0, hi=8)
        for jj in range(8):
            emit_outA(1, Bst, jj + 8, act_stats=True)
            emit_outB(1, Bst, jj, act_stats=True)
        emit_outdma(1, Bst, half=0)
        emit_lnr(1, Bst, act_stats=True, lo=8, hi=NT)
        for jj in range(8, NT):
            emit_outB(1, Bst, jj, act_stats=True)
            if jj == 11:
                emit_outdma(1, Bst, quarter=2)
        emit_outdma(1, Bst, quarter=3)

    nc.finalize()
    return nc


def _get_nc():
    if "nc" not in _CACHE:
        _CACHE["nc"] = _build()
    return _CACHE["nc"]


def _run(x, gamma, beta, trace=False):
    import ml_dtypes

    from concourse.bass_utils import run_bass_kernel_spmd

    x = np.ascontiguousarray(np.asarray(x, dtype=np.float32))
    gamma = np.ascontiguousarray(np.asarray(gamma, dtype=np.float32))
    beta = np.ascontiguousarray(np.asarray(beta, dtype=np.float32))

    xs = x.reshape(N_CORES, NB, T, D)
    xTs = np.ascontiguousarray(xs.transpose(0, 1, 3, 2)).astype(ml_dtypes.bfloat16)

    in_maps = [
        {
            "x": np.ascontiguousarray(xs[c]),
            "xT": xTs[c],
            "gamma": gamma,
            "beta": beta,
        }
        for c in range(N_CORES)
    ]
    res = run_bass_kernel_spmd(
        _get_nc(), in_maps, core_ids=list(range(N_CORES)), trace=trace
    )
    out = np.stack([res.results[c]["out"] for c in range(N_CORES)], axis=0)
    return out.reshape(B, T, D), res


def kernel(x, gamma, beta):
    out, _ = _run(x, gamma, beta, trace=False)
    return out



# revision 3
# speedup vs baseline: 1.1057x; 1.1057x over previous
"""Fused self-attention + residual + LayerNorm kernel for Trainium2.

Reference computation (per batch b of 16):
    S    = x @ x.T                  [2048, 2048]
    A    = softmax(S, axis=-1)
    out  = A @ x                    [2048, 128]
    y    = out + x
    res  = LayerNorm(y) * gamma + beta
Sharding: data-parallel over batch, 2 batches per core on 8 NeuronCores (SPMD,
no collectives).

Algorithm notes (per core / per batch):
  * Stabilized softmax without a max pass: P[q,k] = exp(S[q,k] - (c_q+G)/2)
    with c_q = ||x_q||^2 and G a mid-range constant (soft-max/soft-min of c
    via exp(+-c/6) sums + float-bit ln).  Cauchy-Schwarz bounds the exponent
    by (c_k - G)/2, safely inside bf16/f32 range for G mid-range.
  * No explicit transpose or symmetrization anywhere: the AV matmul uses the
    q-major P tiles directly as lhsT.  Since S is symmetric, tile (j,jj) of P
    equals exp(S[q',k] - (c_k+G)/2) for q' in block jj, k in block j -- so
    with scaled values Vt[k] = t_k x[k] (t_k = exp((c_k-G)/2)) the per-k
    factors cancel: the accumulated result is num'[q'] = t_q' * num[q'].
    The stray t_q' folds into the existing per-row normalization scalar
    R = 1/(den_q * t_q) -- the elementwise P->E multiply of the symmetric
    formulation disappears entirely.
  * exp runs on ACT (bias per-partition, accum_out = row-sum denominators
    for free) for most j-steps; a subset of steps instead computes P on the
    gpsimd engine with a two-op Schraudolph bit-trick
        w = max(S + bias_q, -88);  P_bits(int16) = trunc(A16*w + B16)
    bitcast to bf16 (A16 = 128/ln2).  The clamp keeps the int16 conversion
    out of the NaN band; softmax normalization cancels the ~3% per-entry
    error.  Denominators for those steps come from a DVE row-reduce of P.
    This splits the exp workload across ACT/Pool/DVE so the tensor engine
    (QK^T + AV at their streaming rooflines) becomes the bottleneck.
  * QK^T and AV run in bf16 (f32 PSUM accumulation); rsqrt for LayerNorm is
    fast-inverse-sqrt + 2 Newton steps so ACT stays on one table set.
  * The two batches are software-pipelined: batch 1's main loop overlaps
    batch 0's output stage, and each engine's issue order is time-monotone.
"""

import sys

import numpy as np

sys.path.insert(0, "/opt/trn_rl_repo")

B, T, D = 16, 2048, 128
N_CORES = 8
NB = B // N_CORES          # batches per core
NT = T // 128              # 128-row tiles per batch
EPS = 1e-5

# j-steps whose exp runs on gpsimd (Schraudolph) instead of ACT
SCHR_J = (2, 5, 8, 11, 14)
A16 = 128.0 / 0.6931471805599453        # bf16 Schraudolph scale
B16 = 16251.0                           # 127*128 - minimax shift + trunc comp
LN2 = 0.6931471805599453

_CACHE = {}


def _build():
    from contextlib import ExitStack

    import concourse.bacc as bacc
    import concourse.bass as bass  # noqa: F401
    import concourse.tile as tile
    from concourse import mybir

    f32 = mybir.dt.float32
    bf = mybir.dt.bfloat16
    i16 = mybir.dt.int16
    AF = mybir.ActivationFunctionType
    ALU = mybir.AluOpType
    AX = mybir.AxisListType

    nc = bacc.Bacc()

    x_d = nc.dram_tensor("x", [NB, T, D], f32, kind="ExternalInput")
    xT_d = nc.dram_tensor("xT", [NB, D, T], bf, kind="ExternalInput")
    g_d = nc.dram_tensor("gamma", [D], f32, kind="ExternalInput")
    b_d = nc.dram_tensor("beta", [D], f32, kind="ExternalInput")
    o_d = nc.dram_tensor("out", [NB, T, D], f32, kind="ExternalOutput")

    ctx = ExitStack()
    with tile.TileContext(nc) as tc, ctx:
        big = ctx.enter_context(tc.tile_pool(name="big", bufs=2))
        epool = ctx.enter_context(tc.tile_pool(name="epool", bufs=8))
        stats = ctx.enter_context(tc.tile_pool(name="stats", bufs=2))
        consts = ctx.enter_context(tc.tile_pool(name="consts", bufs=1))
        spool = ctx.enter_context(tc.tile_pool(name="spool", bufs=2, space="PSUM"))
        npool = ctx.enter_context(tc.tile_pool(name="npool", bufs=1, space="PSUM"))

        zero_t = consts.tile([128, 1], f32, tag="zero", name="zero")
        nc.vector.memset(zero_t, 0.0)
        ones_c = consts.tile([128, 1], f32, tag="ones_c", name="ones_c")
        nc.vector.memset(ones_c, 1.0)
        ones_r = consts.tile([1, 128], f32, tag="ones_r", name="ones_r")
        nc.vector.memset(ones_r, 1.0)

        def emit_loads(b, st):
            st["xT"] = big.tile([128, T], bf, tag="xT", name="xT")
            st["x"] = big.tile([128, NT, D], f32, tag="x", name="x")
            xv = x_d[b].rearrange("(t p) d -> p t d", p=128)
            for sx in range(4):
                nc.sync.dma_start(
                    out=st["x"][:, sx * 4 : (sx + 1) * 4, :],
                    in_=xv[:, sx * 4 : (sx + 1) * 4, :],
                )

        def emit_loads_xT(b, st):
            for sx in range(2):
                nc.sync.dma_start(
                    out=st["xT"][:, sx * 1024 : (sx + 1) * 1024],
                    in_=xT_d[b, :, sx * 1024 : (sx + 1) * 1024],
                )

        def emit_stats(b, st):
            x_sb = st["x"]
            C = stats.tile([128, NT], f32, tag="C", name="C")
            sqb = big.tile([128, NT, D], f32, tag="sqb", name="sqb")
            for t in range(NT):
                nc.vector.scalar_tensor_tensor(
                    out=sqb[:, t, :],
                    in0=x_sb[:, t, :],
                    scalar=1.0,
                    in1=x_sb[:, t, :],
                    op0=ALU.mult,
                    op1=ALU.mult,
                    accum_out=C[:, t : t + 1],
                )
            # Soft bounds on the range of c without cross-partition reductions:
            #   cbar = 6 ln(sum exp(c/6))  in [max c, max c + 6 ln 2048]
            #   mbar = -6 ln(sum exp(-c/6)) in [min c - 6 ln 2048, min c]
            # then G = (cbar+mbar)/2 is a mid-range shift.  The ln's use
            # Schraudolph float-bits; cross-partition sum and the broadcast
            # back are K=1/M=2 matmuls on the PE.
            EC = stats.tile([128, NT], f32, tag="EC", name="EC")
            nc.scalar.activation(out=EC, in_=C, func=AF.Exp, bias=zero_t, scale=1.0 / 6.0)
            ECm = stats.tile([128, NT], f32, tag="ECm", name="ECm")
            nc.scalar.activation(out=ECm, in_=C, func=AF.Exp, bias=zero_t, scale=-1.0 / 6.0)
            ec2 = stats.tile([128, 2], f32, tag="ec2", name="ec2")
            nc.vector.tensor_reduce(out=ec2[:, 0:1], in_=EC, axis=AX.X, op=ALU.add)
            nc.vector.tensor_reduce(out=ec2[:, 1:2], in_=ECm, axis=AX.X, op=ALU.add)
            s1a = spool.tile([1, 1], f32, tag="S", name="s1a")
            nc.tensor.matmul(out=s1a, lhsT=ec2[:, 0:1], rhs=ones_c, start=True, stop=True)
            s1b = spool.tile([1, 1], f32, tag="S", name="s1b")
            nc.tensor.matmul(out=s1b, lhsT=ec2[:, 1:2], rhs=ones_c, start=True, stop=True)
            LL2 = stats.tile([1, 2], f32, tag="LL2", name="LL2")
            nc.vector.tensor_copy(out=LL2[0:1, 0:1], in_=s1a.bitcast(mybir.dt.int32))
            nc.vector.tensor_copy(out=LL2[0:1, 1:2], in_=s1b.bitcast(mybir.dt.int32))
            s2 = spool.tile([128, 2], f32, tag="S", name="s2")
            nc.tensor.matmul(out=s2, lhsT=ones_r, rhs=LL2, start=True, stop=True)
            # G = (cbar+mbar)/2 = 3*LN2*(bits_plus - bits_minus)/2^23  [128,1]
            Gd = stats.tile([128, 1], f32, tag="Gd", name="Gd")
            nc.vector.tensor_tensor(
                out=Gd, in0=s2[:, 0:1], in1=s2[:, 1:2], op=ALU.subtract
            )
            Gc = stats.tile([128, 1], f32, tag="Gc", name="Gc")
            nc.vector.tensor_scalar_mul(out=Gc, in0=Gd, scalar1=3.0 * LN2 / 8388608.0)
            Gh_neg = stats.tile([128, 1], f32, tag="Ghn", name="Ghn")
            nc.vector.tensor_scalar_mul(out=Gh_neg, in0=Gc, scalar1=-0.5)
            # bias_all[:, t] = -(c + G)/2
            bias_all = stats.tile([128, NT], f32, tag="bias", name="bias")
            nc.vector.tensor_scalar(
                out=bias_all,
                in0=C,
                scalar1=Gc,
                scalar2=-0.5,
                op0=ALU.add,
                op1=ALU.mult,
            )
            st["bias"] = bias_all
            # t_k = exp((c_k - G)/2); scaled values Vt = t * x (bf16)
            Tall = stats.tile([128, NT], f32, tag="Tall", name="Tall")
            nc.scalar.activation(out=Tall, in_=C, func=AF.Exp, bias=Gh_neg, scale=0.5)
            st["Tall"] = Tall
            Vt = big.tile([128, NT, D], bf, tag="Vt", name="Vt")
            for t in range(NT):
                nc.vector.tensor_scalar_mul(
                    out=Vt[:, t, :], in0=x_sb[:, t, :], scalar1=Tall[:, t : t + 1]
                )
            st["Vt"] = Vt
            st["Den"] = stats.tile([128, NT, 2], f32, tag="Den", name="Den")
            nc.vector.memset(st["Den"], 0.0)

        def emit_main_step(b, st, j):
            if j == 0:
                st["num"] = npool.tile([128, T], f32, tag="num", name="num")
            E_j = epool.tile([128, T], bf, tag="E", name="E")
            xT_sb = st["xT"]
            schr = j in SCHR_J
            for h in range(2):
                S = spool.tile([128, 1024], f32, tag="S", name="S")
                for q in range(2):
                    n0 = h * 1024 + q * 512
                    nc.tensor.matmul(
                        out=S[:, q * 512 : (q + 1) * 512],
                        lhsT=xT_sb[:, j * 128 : (j + 1) * 128],
                        rhs=xT_sb[:, n0 : n0 + 512],
                        start=True,
                        stop=True,
                    )
                if schr:
                    # Pool 2-op Schraudolph: w = max(S+bias,-88);
                    # bits = trunc(A16*w + B16) -> int16 view of bf16 tile
                    tmp = big.tile([128, 1024], f32, tag="stmp", name="stmp")
                    nc.gpsimd.tensor_scalar(
                        out=tmp,
                        in0=S,
                        scalar1=st["bias"][:, j : j + 1],
                        scalar2=-88.0,
                        op0=ALU.add,
                        op1=ALU.max,
                    )
                    nc.gpsimd.tensor_scalar(
                        out=E_j[:, h * 1024 : (h + 1) * 1024].bitcast(i16),
                        in0=tmp,
                        scalar1=A16,
                        scalar2=B16,
                        op0=ALU.mult,
                        op1=ALU.add,
                    )
                else:
                    nc.scalar.activation(
                        out=E_j[:, h * 1024 : (h + 1) * 1024],
                        in_=S,
                        func=AF.Exp,
                        bias=st["bias"][:, j : j + 1],
                        scale=1.0,
                        accum_out=st["Den"][:, j, h : h + 1],
                    )
            if schr:
                nc.vector.tensor_reduce(
                    out=st["Den"][:, j, 0:1], in_=E_j, axis=AX.X, op=ALU.add
                )
            for jj in range(NT):
                # 4 output slices share a 2KB PSUM bank = one zero region:
                # only the bank's first MM sets start, only its last sets stop
                nc.tensor.matmul(
                    out=st["num"][:, jj * 128 : (jj + 1) * 128],
                    lhsT=E_j[:, jj * 128 : (jj + 1) * 128],
                    rhs=st["Vt"][:, j, :],
                    start=(j == 0 and jj % 4 == 0),
                    stop=(j == NT - 1 and jj % 4 == 3),
                )

        def emit_den(b, st):
            den = stats.tile([128, NT], f32, tag="den", name="den")
            nc.vector.tensor_reduce(out=den, in_=st["Den"], axis=AX.X, op=ALU.add)
            denT = stats.tile([128, NT], f32, tag="denT", name="denT")
            nc.vector.tensor_tensor(out=denT, in0=den, in1=st["Tall"], op=ALU.mult)
            R = stats.tile([128, NT], f32, tag="R", name="R")
            nc.vector.reciprocal(out=R, in_=denT)
            st["R"] = R

        def emit_drain(b, st, copy_psum=True, half=None):
            # drain AV results out of PSUM so the next batch can reuse it
            # (skipped for the last batch -- nothing needs the banks).
            # Staggered: half 0 at the phase boundary, half 1 a few iterations
            # later, so the copies don't starve DVE mid-phase.
            if copy_psum:
                if half != 1:
                    st["numS"] = big.tile([128, T], f32, tag="numS", name="numS")
                for h in ([0, 1] if half is None else [half]):
                    nc.vector.tensor_copy(
                        out=st["numS"][:, h * 1024 : (h + 1) * 1024],
                        in_=st["num"][:, h * 1024 : (h + 1) * 1024],
                    )
            else:
                st["numS"] = st["num"]
            if half != 1:
                st["Y"] = big.tile([128, NT, D], f32, tag="Y", name="Y")
                st["MV"] = stats.tile([128, NT, 2], f32, tag="MV", name="MV")
                st["Yout"] = big.tile([128, NT, D], f32, tag="Yout", name="Yout")

        def emit_outA(b, st, jj, act_stats=False):
            # y = num'/den' + x.  LN stats either via DVE bn_stats (b0: DVE has
            # slack mid-phase) or via accum_out (b1 tail: split DVE/Pool).
            eng = nc.gpsimd if (act_stats and jj % 2 == 1) else nc.vector
            eng.scalar_tensor_tensor(
                out=st["Y"][:, jj, :],
                in0=st["numS"][:, jj * 128 : (jj + 1) * 128],
                scalar=st["R"][:, jj : jj + 1],
                in1=st["x"][:, jj, :],
                op0=ALU.mult,
                op1=ALU.add,
                accum_out=st["Sy"][:, jj : jj + 1] if act_stats else None,
            )
            if act_stats:
                eng.scalar_tensor_tensor(
                    out=st["ysqb"][:, jj, :],
                    in0=st["Y"][:, jj, :],
                    scalar=1.0,
                    in1=st["Y"][:, jj, :],
                    op0=ALU.mult,
                    op1=ALU.mult,
                    accum_out=st["Sy2"][:, jj : jj + 1],
                )
            else:
                bns = stats.tile([128, 6], f32, tag="bns", name="bns")
                nc.vector.bn_stats(out=bns, in_=st["Y"][:, jj, :])
                nc.vector.bn_aggr(out=st["MV"][:, jj, :], in_=bns)

        def emit_lnr(b, st, act_stats=False, lo=0, hi=NT):
            cs = slice(lo, hi)
            if act_stats:
                if "mu" not in st:
                    st["mu"] = stats.tile([128, NT], f32, tag="mu", name="mu")
                    st["vart"] = stats.tile([128, NT], f32, tag="vart", name="vart")
                    st["rstd"] = stats.tile([128, NT], f32, tag="rstd", name="rstd")
                # mu = Sy/128, var = Sy2/128 - mu^2
                nc.vector.tensor_scalar_mul(
                    out=st["mu"][:, cs], in0=st["Sy"][:, cs], scalar1=1.0 / D
                )
                musq = stats.tile([128, NT], f32, tag="musq", name="musq")
                nc.vector.scalar_tensor_tensor(
                    out=musq[:, cs],
                    in0=st["mu"][:, cs],
                    scalar=1.0,
                    in1=st["mu"][:, cs],
                    op0=ALU.mult,
                    op1=ALU.mult,
                )
                nc.vector.scalar_tensor_tensor(
                    out=st["vart"][:, cs],
                    in0=st["Sy2"][:, cs],
                    scalar=1.0 / D,
                    in1=musq[:, cs],
                    op0=ALU.mult,
                    op1=ALU.subtract,
                )
                var_in = st["vart"][:, cs]
            else:
                if "rstd" not in st:
                    st["rstd"] = stats.tile([128, NT], f32, tag="rstd", name="rstd")
                var_in = st["MV"][:, cs, 1]
            # rstd = 1/sqrt(var+eps) via the fast-inverse-sqrt bit trick plus
            # two Newton steps (~4e-6 rel err) -- keeps the ACT engine on the
            # exp table set for the whole kernel (table swaps cost 1.3us each)
            ve = stats.tile([128, NT], f32, tag="ve", name="ve")
            nc.vector.tensor_scalar_add(out=ve[:, cs], in0=var_in, scalar1=EPS)
            wf = stats.tile([128, NT], f32, tag="wf", name="wf")
            nc.vector.tensor_copy(out=wf[:, cs], in_=ve[:, cs].bitcast(mybir.dt.int32))
            nc.vector.tensor_scalar(
                out=wf[:, cs], in0=wf[:, cs],
                scalar1=-0.5, scalar2=1597463007.0,
                op0=ALU.mult, op1=ALU.add,
            )
            wi = stats.tile([128, NT], mybir.dt.int32, tag="wi", name="wi")
            nc.vector.tensor_copy(out=wi[:, cs], in_=wf[:, cs])
            y = stats.tile([128, NT], f32, tag="y0", name="y0")
            nc.vector.tensor_copy(out=y[:, cs], in_=wi[:, cs].bitcast(f32))
            t1 = stats.tile([128, NT], f32, tag="t1", name="t1")
            for _ in range(2):
                nc.vector.tensor_mul(out=t1[:, cs], in0=ve[:, cs], in1=y[:, cs])
                nc.vector.tensor_mul(out=t1[:, cs], in0=t1[:, cs], in1=y[:, cs])
                nc.vector.tensor_scalar(
                    out=t1[:, cs], in0=t1[:, cs],
                    scalar1=-0.5, scalar2=1.5, op0=ALU.mult, op1=ALU.add,
                )
                nc.vector.tensor_mul(out=y[:, cs], in0=y[:, cs], in1=t1[:, cs])
            nc.vector.tensor_copy(out=st["rstd"][:, cs], in_=y[:, cs])

        def emit_outB(b, st, jj, act_stats=False):
            mu_s = st["mu"][:, jj : jj + 1] if act_stats else st["MV"][:, jj, 0:1]
            z = stats.tile([128, D], f32, tag="z", name="z")
            zeng = nc.gpsimd if (act_stats and jj % 2 == 1) else nc.vector
            zeng.tensor_scalar(
                out=z,
                in0=st["Y"][:, jj, :],
                scalar1=mu_s,
                scalar2=st["rstd"][:, jj : jj + 1],
                op0=ALU.subtract,
                op1=ALU.mult,
            )
            z2 = stats.tile([128, D], f32, tag="z2", name="z2")
            nc.gpsimd.tensor_mul(out=z2, in0=z, in1=gb)
            nc.gpsimd.tensor_add(out=st["Yout"][:, jj, :], in0=z2, in1=bb)

        def emit_outdma(b, st, half=None, quarter=None):
            ov = o_d[b].rearrange("(t p) d -> p t d", p=128)
            if quarter is not None:
                q4 = slice(quarter * 4, (quarter + 1) * 4)
                nc.sync.dma_start(out=ov[:, q4, :], in_=st["Yout"][:, q4, :])
            elif half is None:
                nc.sync.dma_start(out=ov, in_=st["Yout"])
            else:
                h8 = slice(half * 8, (half + 1) * 8)
                nc.sync.dma_start(out=ov[:, h8, :], in_=st["Yout"][:, h8, :])

        # ---- software-pipelined schedule over the two batches ---------------
        A, Bst = {}, {}
        emit_loads(0, A)
        emit_stats(0, A)
        emit_loads_xT(0, A)
        emit_loads(1, Bst)
        emit_loads_xT(1, Bst)
        gb = consts.tile([128, D], f32, tag="gb", name="gb")
        bb = consts.tile([128, D], f32, tag="bb", name="bb")
        for j in range(NT):
            emit_main_step(0, A, j)
            if j == 3:
                emit_stats(1, Bst)
            if j == 5:
                nc.sync.dma_start(out=gb, in_=g_d[:].partition_broadcast(128))
                nc.sync.dma_start(out=bb, in_=b_d[:].partition_broadcast(128))
        emit_den(0, A)
        emit_drain(0, A, half=0)
        # phase 1: batch 1's main loop with batch 0's whole output stage
        # threaded through it (outA x2 in early iters, lnr at 8, outB x2 late)
        for j in range(NT):
            emit_main_step(1, Bst, j)
            if j == 2:
                emit_drain(0, A, half=1)
            if j < 8:
                emit_outA(0, A, 2 * j)
                emit_outA(0, A, 2 * j + 1)
            else:
                if j == 8:
                    emit_lnr(0, A)
                emit_outB(0, A, 2 * (j - 8))
                emit_outB(0, A, 2 * (j - 8) + 1)
                if j == 12:
                    emit_outdma(0, A, half=0)
        emit_outdma(0, A, half=1)
        emit_den(1, Bst)
        emit_drain(1, Bst, copy_psum=False)
        Bst["Sy"] = stats.tile([128, NT], f32, tag="Sy", name="Sy")
        Bst["Sy2"] = stats.tile([128, NT], f32, tag="Sy2", name="Sy2")
        Bst["ysqb"] = big.tile([128, NT, D], f32, tag="ysqb", name="ysqb", bufs=1)
        # half-split tail: LN stats for tiles 0-7 finish while 8-15 are still
        # accumulating, so normalize+store of the first half overlaps the rest
        for jj in range(8):
            emit_outA(1, Bst, jj, act_stats=True)
        emit_lnr(1, Bst, act_stats=True, lo=0, hi=8)
        for jj in range(8):
            emit_outA(1, Bst, jj + 8, act_stats=True)
            emit_outB(1, Bst, jj, act_stats=True)
        emit_outdma(1, Bst, half=0)
        emit_lnr(1, Bst, act_stats=True, lo=8, hi=NT)
        for jj in range(8, NT):
            emit_outB(1, Bst, jj, act_stats=True)
            if jj == 11:
                emit_outdma(1, Bst, quarter=2)
        emit_outdma(1, Bst, quarter=3)

    nc.finalize()
    return nc


def _get_nc():
    if "nc" not in _CACHE:
        _CACHE["nc"] = _build()
    return _CACHE["nc"]


def _run(x, gamma, beta, trace=False):
    import ml_dtypes

    from concourse.bass_utils import run_bass_kernel_spmd

    x = np.ascontiguousarray(np.asarray(x, dtype=np.float32))
    gamma = np.ascontiguousarray(np.asarray(gamma, dtype=np.float32))
    beta = np.ascontiguousarray(np.asarray(beta, dtype=np.float32))

    xs = x.reshape(N_CORES, NB, T, D)
    xTs = np.ascontiguousarray(xs.transpose(0, 1, 3, 2)).astype(ml_dtypes.bfloat16)

    in_maps = [
        {
            "x": np.ascontiguousarray(xs[c]),
            "xT": xTs[c],
            "gamma": gamma,
            "beta": beta,
        }
        for c in range(N_CORES)
    ]
    res = run_bass_kernel_spmd(
        _get_nc(), in_maps, core_ids=list(range(N_CORES)), trace=trace
    )
    out = np.stack([res.results[c]["out"] for c in range(N_CORES)], axis=0)
    return out.reshape(B, T, D), res


def kernel(x, gamma, beta):
    out, _ = _run(x, gamma, beta, trace=False)
    return out


# revision 9
# speedup vs baseline: 1.1799x; 1.0671x over previous
"""Fused self-attention + residual + LayerNorm kernel for Trainium2.

Reference computation (per batch b of 16):
    S    = x @ x.T                  [2048, 2048]
    A    = softmax(S, axis=-1)
    out  = A @ x                    [2048, 128]
    y    = out + x
    res  = LayerNorm(y) * gamma + beta
Sharding: data-parallel over batch, 2 batches per core on 8 NeuronCores (SPMD,
no collectives).

Algorithm notes (per core / per batch):
  * Stabilized softmax without a max pass: P[q,k] = exp(S[q,k] - (c_q+G)/2)
    with c_q = ||x_q||^2 and G a mid-range constant (soft-max/soft-min of c
    via exp(+-c/6) sums + float-bit ln).  Cauchy-Schwarz bounds the exponent
    by (c_k - G)/2, safely inside bf16/f32 range for G mid-range.
  * No explicit transpose or symmetrization anywhere: the AV matmul uses the
    q-major P tiles directly as lhsT.  Since S is symmetric, tile (j,jj) of P
    equals exp(S[q',k] - (c_k+G)/2) for q' in block jj, k in block j -- so
    with scaled values Vt[k] = t_k x[k] (t_k = exp((c_k-G)/2)) the per-k
    factors cancel: the accumulated result is num'[q'] = t_q' * num[q'].
    The stray t_q' folds into the existing per-row normalization scalar
    R = 1/(den_q * t_q) -- the elementwise P->E multiply of the symmetric
    formulation disappears entirely.
  * exp runs on ACT (bias per-partition, accum_out = row-sum denominators
    for free) for most j-steps; a subset of steps instead computes P on the
    gpsimd engine with a two-op Schraudolph bit-trick
        w = max(S + bias_q, -88);  P_bits(int16) = trunc(A16*w + B16)
    bitcast to bf16 (A16 = 128/ln2).  The clamp keeps the int16 conversion
    out of the NaN band; softmax normalization cancels the ~3% per-entry
    error.  Denominators for those steps come from a DVE row-reduce of P.
    This splits the exp workload across ACT/Pool/DVE so the tensor engine
    (QK^T + AV at their streaming rooflines) becomes the bottleneck.
  * QK^T and AV run in bf16 (f32 PSUM accumulation); rsqrt for LayerNorm is
    fast-inverse-sqrt + 2 Newton steps so ACT stays on one table set.
  * The two batches are software-pipelined: batch 1's main loop overlaps
    batch 0's output stage, and each engine's issue order is time-monotone.
"""

import sys

import numpy as np

sys.path.insert(0, "/opt/trn_rl_repo")

B, T, D = 16, 2048, 128
N_CORES = 8
NB = B // N_CORES          # batches per core
NT = T // 128              # 128-row tiles per batch
EPS = 1e-5

# per-j engines for the two 1024-wide exp halves: 'A' = ACT LUT exp,
# 'P' = gpsimd two-op Schraudolph.  Denominators for 'P' halves come from a
# DVE row-reduce of the bf16 P half-tile (full-tile AP -> 2x bf16 mode).
MODE = {j: ("A", "A") for j in range(16)}
for j in (2, 5, 8, 11, 14):
    MODE[j] = ("P", "P")
MODE[7] = ("A", "P")
A16 = 128.0 / 0.6931471805599453        # bf16 Schraudolph scale
B16 = 16251.0                           # 127*128 - minimax shift + trunc comp
LN2 = 0.6931471805599453

_CACHE = {}


def _build():
    from contextlib import ExitStack

    import concourse.bacc as bacc
    import concourse.bass as bass  # noqa: F401
    import concourse.tile as tile
    from concourse import mybir

    f32 = mybir.dt.float32
    bf = mybir.dt.bfloat16
    i16 = mybir.dt.int16
    AF = mybir.ActivationFunctionType
    ALU = mybir.AluOpType
    AX = mybir.AxisListType

    nc = bacc.Bacc()

    x_d = nc.dram_tensor("x", [NB, T, D], f32, kind="ExternalInput")
    xT_d = nc.dram_tensor("xT", [NB, D, T], bf, kind="ExternalInput")
    g_d = nc.dram_tensor("gamma", [D], f32, kind="ExternalInput")
    b_d = nc.dram_tensor("beta", [D], f32, kind="ExternalInput")
    o_d = nc.dram_tensor("out", [NB, T, D], f32, kind="ExternalOutput")

    ctx = ExitStack()
    with tile.TileContext(nc) as tc, ctx:
        big = ctx.enter_context(tc.tile_pool(name="big", bufs=2))
        epool = ctx.enter_context(tc.tile_pool(name="epool", bufs=8))
        stats = ctx.enter_context(tc.tile_pool(name="stats", bufs=2))
        consts = ctx.enter_context(tc.tile_pool(name="consts", bufs=1))
        spool = ctx.enter_context(tc.tile_pool(name="spool", bufs=2, space="PSUM"))
        npool = ctx.enter_context(tc.tile_pool(name="npool", bufs=1, space="PSUM"))

        zero_t = consts.tile([128, 1], f32, tag="zero", name="zero")
        nc.vector.memset(zero_t, 0.0)
        ones_c = consts.tile([128, 1], f32, tag="ones_c", name="ones_c")
        nc.vector.memset(ones_c, 1.0)
        ones_r = consts.tile([1, 128], f32, tag="ones_r", name="ones_r")
        nc.vector.memset(ones_r, 1.0)

        def emit_loads(b, st):
            st["xT"] = big.tile([128, T], bf, tag="xT", name="xT")
            st["x"] = big.tile([128, NT, D], f32, tag="x", name="x")
            xv = x_d[b].rearrange("(t p) d -> p t d", p=128)
            for sx in range(4):
                nc.sync.dma_start(
                    out=st["x"][:, sx * 4 : (sx + 1) * 4, :],
                    in_=xv[:, sx * 4 : (sx + 1) * 4, :],
                )

        def emit_loads_xT(b, st):
            for sx in range(2):
                nc.sync.dma_start(
                    out=st["xT"][:, sx * 1024 : (sx + 1) * 1024],
                    in_=xT_d[b, :, sx * 1024 : (sx + 1) * 1024],
                )

        def emit_stats(b, st):
            x_sb = st["x"]
            C = stats.tile([128, NT], f32, tag="C", name="C")
            sqb = big.tile([128, NT, D], f32, tag="sqb", name="sqb")
            for t in range(NT):
                nc.gpsimd.scalar_tensor_tensor(
                    out=sqb[:, t, :],
                    in0=x_sb[:, t, :],
                    scalar=1.0,
                    in1=x_sb[:, t, :],
                    op0=ALU.mult,
                    op1=ALU.mult,
                    accum_out=C[:, t : t + 1],
                )
            # Soft bounds on the range of c without cross-partition reductions:
            #   cbar = 6 ln(sum exp(c/6))  in [max c, max c + 6 ln 2048]
            #   mbar = -6 ln(sum exp(-c/6)) in [min c - 6 ln 2048, min c]
            # then G = (cbar+mbar)/2 is a mid-range shift.  The ln's use
            # Schraudolph float-bits; cross-partition sum and the broadcast
            # back are K=1/M=2 matmuls on the PE.
            EC = stats.tile([128, NT], f32, tag="EC", name="EC")
            nc.scalar.activation(out=EC, in_=C, func=AF.Exp, bias=zero_t, scale=1.0 / 6.0)
            ECm = stats.tile([128, NT], f32, tag="ECm", name="ECm")
            nc.scalar.activation(out=ECm, in_=C, func=AF.Exp, bias=zero_t, scale=-1.0 / 6.0)
            ec2 = stats.tile([128, 2], f32, tag="ec2", name="ec2")
            nc.vector.tensor_reduce(out=ec2[:, 0:1], in_=EC, axis=AX.X, op=ALU.add)
            nc.vector.tensor_reduce(out=ec2[:, 1:2], in_=ECm, axis=AX.X, op=ALU.add)
            s1a = spool.tile([1, 1], f32, tag="S", name="s1a")
            nc.tensor.matmul(out=s1a, lhsT=ec2[:, 0:1], rhs=ones_c, start=True, stop=True)
            s1b = spool.tile([1, 1], f32, tag="S", name="s1b")
            nc.tensor.matmul(out=s1b, lhsT=ec2[:, 1:2], rhs=ones_c, start=True, stop=True)
            LL2 = stats.tile([1, 2], f32, tag="LL2", name="LL2")
            nc.vector.tensor_copy(out=LL2[0:1, 0:1], in_=s1a.bitcast(mybir.dt.int32))
            nc.vector.tensor_copy(out=LL2[0:1, 1:2], in_=s1b.bitcast(mybir.dt.int32))
            s2 = spool.tile([128, 2], f32, tag="S", name="s2")
            nc.tensor.matmul(out=s2, lhsT=ones_r, rhs=LL2, start=True, stop=True)
            # G = (cbar+mbar)/2 = 3*LN2*(bits_plus - bits_minus)/2^23  [128,1]
            s2s = stats.tile([128, 2], f32, tag="s2s", name="s2s")
            nc.vector.tensor_copy(out=s2s, in_=s2)
            Gd = stats.tile([128, 1], f32, tag="Gd", name="Gd")
            nc.vector.tensor_tensor(
                out=Gd, in0=s2s[:, 0:1], in1=s2s[:, 1:2], op=ALU.subtract
            )
            Gc = stats.tile([128, 1], f32, tag="Gc", name="Gc")
            nc.vector.tensor_scalar_mul(out=Gc, in0=Gd, scalar1=3.0 * LN2 / 8388608.0)
            Gh_neg = stats.tile([128, 1], f32, tag="Ghn", name="Ghn")
            nc.vector.tensor_scalar_mul(out=Gh_neg, in0=Gc, scalar1=-0.5)
            # bias_all[:, t] = -(c + G)/2
            bias_all = stats.tile([128, NT], f32, tag="bias", name="bias")
            nc.vector.tensor_scalar(
                out=bias_all,
                in0=C,
                scalar1=Gc,
                scalar2=-0.5,
                op0=ALU.add,
                op1=ALU.mult,
            )
            st["bias"] = bias_all
            # t_k = exp((c_k - G)/2); scaled values Vt = t * x (bf16)
            Tall = stats.tile([128, NT], f32, tag="Tall", name="Tall")
            nc.scalar.activation(out=Tall, in_=C, func=AF.Exp, bias=Gh_neg, scale=0.5)
            st["Tall"] = Tall
            Vt = big.tile([128, NT, D], bf, tag="Vt", name="Vt")
            for t in range(NT):
                nc.gpsimd.tensor_scalar_mul(
                    out=Vt[:, t, :], in0=x_sb[:, t, :], scalar1=Tall[:, t : t + 1]
                )
            st["Vt"] = Vt
            st["Den"] = stats.tile([128, NT, 2], f32, tag="Den", name="Den")
            nc.vector.memset(st["Den"], 0.0)

        def emit_qk_exp(b, st, j):
            # QK^T for row-block j + exp into two bf16 half-tiles (full-tile
            # APs so DVE reduces get the 2x bf16 mode).  The AV matmuls for
            # block j are emitted one step later (emit_av) so PE work overlaps
            # the exp of the next block instead of gating it.
            if j == 0:
                st["num"] = npool.tile([128, T], f32, tag="num", name="num")
                st["E"] = {}
            Eh = [
                epool.tile([128, 1024], bf, tag="E0", name="E0"),
                epool.tile([128, 1024], bf, tag="E1", name="E1"),
            ]
            st["E"][j] = Eh
            xT_sb = st["xT"]
            for h in range(2):
                S = spool.tile([128, 1024], f32, tag="S", name="S")
                for q in range(2):
                    n0 = h * 1024 + q * 512
                    nc.tensor.matmul(
                        out=S[:, q * 512 : (q + 1) * 512],
                        lhsT=xT_sb[:, j * 128 : (j + 1) * 128],
                        rhs=xT_sb[:, n0 : n0 + 512],
                        start=True,
                        stop=True,
                    )
                if MODE[j][h] == "P":
                    # Pool 2-op Schraudolph: w = max(S+bias,-88);
                    # bits = trunc(A16*w + B16) -> int16 view of bf16 tile
                    tmp = big.tile([128, 1024], f32, tag="stmp", name="stmp")
                    nc.gpsimd.tensor_scalar(
                        out=tmp,
                        in0=S,
                        scalar1=st["bias"][:, j : j + 1],
                        scalar2=-88.0,
                        op0=ALU.add,
                        op1=ALU.max,
                    )
                    nc.gpsimd.tensor_scalar(
                        out=Eh[h].bitcast(i16),
                        in0=tmp,
                        scalar1=A16,
                        scalar2=B16,
                        op0=ALU.mult,
                        op1=ALU.add,
                    )
                    nc.vector.tensor_reduce(
                        out=st["Den"][:, j, h : h + 1], in_=Eh[h], axis=AX.X,
                        op=ALU.add,
                    )
                else:
                    nc.scalar.activation(
                        out=Eh[h],
                        in_=S,
                        func=AF.Exp,
                        bias=st["bias"][:, j : j + 1],
                        scale=1.0,
                        accum_out=st["Den"][:, j, h : h + 1],
                    )

        def emit_av(b, st, j):
            Eh = st["E"].pop(j)
            for jj in range(NT):
                # 4 output slices share a 2KB PSUM bank = one zero region:
                # only the bank's first MM sets start, only its last sets stop
                nc.tensor.matmul(
                    out=st["num"][:, jj * 128 : (jj + 1) * 128],
                    lhsT=Eh[jj // 8][:, (jj % 8) * 128 : (jj % 8 + 1) * 128],
                    rhs=st["Vt"][:, j, :],
                    start=(j == 0 and jj % 4 == 0),
                    stop=(j == NT - 1 and jj % 4 == 3),
                )

        def emit_den(b, st):
            den = stats.tile([128, NT], f32, tag="den", name="den")
            nc.vector.tensor_reduce(out=den, in_=st["Den"], axis=AX.X, op=ALU.add)
            denT = stats.tile([128, NT], f32, tag="denT", name="denT")
            nc.vector.tensor_tensor(out=denT, in0=den, in1=st["Tall"], op=ALU.mult)
            R = stats.tile([128, NT], f32, tag="R", name="R")
            nc.vector.reciprocal(out=R, in_=denT)
            st["R"] = R

        def emit_drain(b, st, copy_psum=True, half=None):
            # drain AV results out of PSUM so the next batch can reuse it
            # (skipped for the last batch -- nothing needs the banks).
            # Staggered: half 0 at the phase boundary, half 1 a few iterations
            # later, so the copies don't starve DVE mid-phase.
            if copy_psum:
                if half != 1:
                    st["numS"] = big.tile([128, T], f32, tag="numS", name="numS")
                for h in ([0, 1] if half is None else [half]):
                    nc.vector.tensor_copy(
                        out=st["numS"][:, h * 1024 : (h + 1) * 1024],
                        in_=st["num"][:, h * 1024 : (h + 1) * 1024],
                    )
            else:
                st["numS"] = st["num"]
            if half != 1:
                st["Y"] = big.tile([128, NT, D], f32, tag="Y", name="Y")
                st["MV"] = stats.tile([128, NT, 2], f32, tag="MV", name="MV")
                st["Yout"] = big.tile([128, NT, D], f32, tag="Yout", name="Yout")

        def emit_outA(b, st, jj, act_stats=False):
            # y = num'/den' + x.  LN stats either via DVE bn_stats (b0: DVE has
            # slack mid-phase) or via accum_out (b1 tail: split DVE/Pool).
            eng = nc.gpsimd if (act_stats and jj % 2 == 1) else nc.vector
            eng.scalar_tensor_tensor(
                out=st["Y"][:, jj, :],
                in0=st["numS"][:, jj * 128 : (jj + 1) * 128],
                scalar=st["R"][:, jj : jj + 1],
                in1=st["x"][:, jj, :],
                op0=ALU.mult,
                op1=ALU.add,
                accum_out=st["Sy"][:, jj : jj + 1] if act_stats else None,
            )
            if act_stats:
                eng.scalar_tensor_tensor(
                    out=st["ysqb"][:, jj, :],
                    in0=st["Y"][:, jj, :],
                    scalar=1.0,
                    in1=st["Y"][:, jj, :],
                    op0=ALU.mult,
                    op1=ALU.mult,
                    accum_out=st["Sy2"][:, jj : jj + 1],
                )
            else:
                bns = stats.tile([128, 6], f32, tag="bns", name="bns")
                nc.vector.bn_stats(out=bns, in_=st["Y"][:, jj, :])
                nc.vector.bn_aggr(out=st["MV"][:, jj, :], in_=bns)

        def emit_lnr(b, st, act_stats=False, lo=0, hi=NT):
            cs = slice(lo, hi)
            if act_stats:
                if "mu" not in st:
                    st["mu"] = stats.tile([128, NT], f32, tag="mu", name="mu")
                    st["vart"] = stats.tile([128, NT], f32, tag="vart", name="vart")
                    st["rstd"] = stats.tile([128, NT], f32, tag="rstd", name="rstd")
                # mu = Sy/128, var = Sy2/128 - mu^2
                nc.vector.tensor_scalar_mul(
                    out=st["mu"][:, cs], in0=st["Sy"][:, cs], scalar1=1.0 / D
                )
                musq = stats.tile([128, NT], f32, tag="musq", name="musq")
                nc.vector.scalar_tensor_tensor(
                    out=musq[:, cs],
                    in0=st["mu"][:, cs],
                    scalar=1.0,
                    in1=st["mu"][:, cs],
                    op0=ALU.mult,
                    op1=ALU.mult,
                )
                nc.vector.scalar_tensor_tensor(
                    out=st["vart"][:, cs],
                    in0=st["Sy2"][:, cs],
                    scalar=1.0 / D,
                    in1=musq[:, cs],
                    op0=ALU.mult,
                    op1=ALU.subtract,
                )
                var_in = st["vart"][:, cs]
            else:
                if "rstd" not in st:
                    st["rstd"] = stats.tile([128, NT], f32, tag="rstd", name="rstd")
                var_in = st["MV"][:, cs, 1]
            # rstd = 1/sqrt(var+eps) via the fast-inverse-sqrt bit trick plus
            # two Newton steps (~4e-6 rel err) -- keeps the ACT engine on the
            # exp table set for the whole kernel (table swaps cost 1.3us each)
            ve = stats.tile([128, NT], f32, tag="ve", name="ve")
            nc.vector.tensor_scalar_add(out=ve[:, cs], in0=var_in, scalar1=EPS)
            wf = stats.tile([128, NT], f32, tag="wf", name="wf")
            nc.vector.tensor_copy(out=wf[:, cs], in_=ve[:, cs].bitcast(mybir.dt.int32))
            nc.vector.tensor_scalar(
                out=wf[:, cs], in0=wf[:, cs],
                scalar1=-0.5, scalar2=1597463007.0,
                op0=ALU.mult, op1=ALU.add,
            )
            wi = stats.tile([128, NT], mybir.dt.int32, tag="wi", name="wi")
            nc.vector.tensor_copy(out=wi[:, cs], in_=wf[:, cs])
            y = stats.tile([128, NT], f32, tag="y0", name="y0")
            nc.vector.tensor_copy(out=y[:, cs], in_=wi[:, cs].bitcast(f32))
            t1 = stats.tile([128, NT], f32, tag="t1", name="t1")
            for _ in range(2):
                nc.vector.tensor_mul(out=t1[:, cs], in0=ve[:, cs], in1=y[:, cs])
                nc.vector.tensor_mul(out=t1[:, cs], in0=t1[:, cs], in1=y[:, cs])
                nc.vector.tensor_scalar(
                    out=t1[:, cs], in0=t1[:, cs],
                    scalar1=-0.5, scalar2=1.5, op0=ALU.mult, op1=ALU.add,
                )
                nc.vector.tensor_mul(out=y[:, cs], in0=y[:, cs], in1=t1[:, cs])
            nc.vector.tensor_copy(out=st["rstd"][:, cs], in_=y[:, cs])

        def emit_outB(b, st, jj, act_stats=False):
            mu_s = st["mu"][:, jj : jj + 1] if act_stats else st["MV"][:, jj, 0:1]
            z = stats.tile([128, D], f32, tag="z", name="z")
            zeng = nc.gpsimd if (act_stats and jj % 2 == 1) else nc.vector
            zeng.tensor_scalar(
                out=z,
                in0=st["Y"][:, jj, :],
                scalar1=mu_s,
                scalar2=st["rstd"][:, jj : jj + 1],
                op0=ALU.subtract,
                op1=ALU.mult,
            )
            z2 = stats.tile([128, D], f32, tag="z2", name="z2")
            nc.gpsimd.tensor_mul(out=z2, in0=z, in1=gb)
            nc.gpsimd.tensor_add(out=st["Yout"][:, jj, :], in0=z2, in1=bb)

        def emit_outdma(b, st, half=None, quarter=None):
            ov = o_d[b].rearrange("(t p) d -> p t d", p=128)
            if quarter is not None:
                q4 = slice(quarter * 4, (quarter + 1) * 4)
                nc.sync.dma_start(out=ov[:, q4, :], in_=st["Yout"][:, q4, :])
            elif half is None:
                nc.sync.dma_start(out=ov, in_=st["Yout"])
            else:
                h8 = slice(half * 8, (half + 1) * 8)
                nc.sync.dma_start(out=ov[:, h8, :], in_=st["Yout"][:, h8, :])

        # ---- software-pipelined schedule over the two batches ---------------
        A, Bst = {}, {}
        emit_loads(0, A)
        emit_stats(0, A)
        emit_loads_xT(0, A)
        emit_loads(1, Bst)
        emit_loads_xT(1, Bst)
        gb = consts.tile([128, D], f32, tag="gb", name="gb")
        bb = consts.tile([128, D], f32, tag="bb", name="bb")
        for j in range(NT):
            emit_qk_exp(0, A, j)
            if j > 0:
                emit_av(0, A, j - 1)
            if j == 3:
                emit_stats(1, Bst)
            if j == 5:
                nc.sync.dma_start(out=gb, in_=g_d[:].partition_broadcast(128))
                nc.sync.dma_start(out=bb, in_=b_d[:].partition_broadcast(128))
        emit_av(0, A, NT - 1)
        emit_den(0, A)
        emit_drain(0, A, half=0)
        # phase 1: batch 1's main loop with batch 0's whole output stage
        # threaded through it (outA x2 in early iters, lnr at 8, outB x2 late)
        for j in range(NT):
            emit_qk_exp(1, Bst, j)
            if j > 0:
                emit_av(1, Bst, j - 1)
            if j == 2:
                emit_drain(0, A, half=1)
            if j < 8:
                emit_outA(0, A, 2 * j)
                emit_outA(0, A, 2 * j + 1)
            else:
                if j == 8:
                    emit_lnr(0, A)
                emit_outB(0, A, 2 * (j - 8))
                emit_outB(0, A, 2 * (j - 8) + 1)
                if j == 12:
                    emit_outdma(0, A, half=0)
        emit_av(1, Bst, NT - 1)
        emit_outdma(0, A, half=1)
        emit_den(1, Bst)
        emit_drain(1, Bst, copy_psum=False)
        Bst["Sy"] = stats.tile([128, NT], f32, tag="Sy", name="Sy")
        Bst["Sy2"] = stats.tile([128, NT], f32, tag="Sy2", name="Sy2")
        Bst["ysqb"] = big.tile([128, NT, D], f32, tag="ysqb", name="ysqb", bufs=1)
        # half-split tail: LN stats for tiles 0-7 finish while 8-15 are still
        # accumulating, so normalize+store of the first half overlaps the rest
        for jj in range(8):
            emit_outA(1, Bst, jj, act_stats=True)
        emit_lnr(1, Bst, act_stats=True, lo=0, hi=8)
        for jj in range(8):
            emit_outA(1, Bst, jj + 8, act_stats=True)
            emit_outB(1, Bst, jj, act_stats=True)
        emit_outdma(1, Bst, half=0)
        emit_lnr(1, Bst, act_stats=True, lo=8, hi=NT)
        for jj in range(8, NT):
            emit_outB(1, Bst, jj, act_stats=True)
            if jj == 11:
                emit_outdma(1, Bst, quarter=2)
        emit_outdma(1, Bst, quarter=3)

    nc.finalize()
    return nc


def _get_nc():
    if "nc" not in _CACHE:
        _CACHE["nc"] = _build()
    return _CACHE["nc"]


def _run(x, gamma, beta, trace=False):
    import ml_dtypes

    from concourse.bass_utils import run_bass_kernel_spmd

    x = np.ascontiguousarray(np.asarray(x, dtype=np.float32))
    gamma = np.ascontiguousarray(np.asarray(gamma, dtype=np.float32))
    beta = np.ascontiguousarray(np.asarray(beta, dtype=np.float32))

    xs = x.reshape(N_CORES, NB, T, D)
    xTs = np.ascontiguousarray(xs.transpose(0, 1, 3, 2)).astype(ml_dtypes.bfloat16)

    in_maps = [
        {
            "x": np.ascontiguousarray(xs[c]),
            "xT": xTs[c],
            "gamma": gamma,
            "beta": beta,
        }
        for c in range(N_CORES)
    ]
    res = run_bass_kernel_spmd(
        _get_nc(), in_maps, core_ids=list(range(N_CORES)), trace=trace
    )
    out = np.stack([res.results[c]["out"] for c in range(N_CORES)], axis=0)
    return out.reshape(B, T, D), res


def kernel(x, gamma, beta):
    out, _ = _run(x, gamma, beta, trace=False)
    return out


# revision 11
# speedup vs baseline: 1.1802x; 1.0002x over previous
"""Fused self-attention + residual + LayerNorm kernel for Trainium2.

Reference computation (per batch b of 16):
    S    = x @ x.T                  [2048, 2048]
    A    = softmax(S, axis=-1)
    out  = A @ x                    [2048, 128]
    y    = out + x
    res  = LayerNorm(y) * gamma + beta
Sharding: data-parallel over batch, 2 batches per core on 8 NeuronCores (SPMD,
no collectives).

Algorithm notes (per core / per batch):
  * Stabilized softmax without a max pass: P[q,k] = exp(S[q,k] - (c_q+G)/2)
    with c_q = ||x_q||^2 and G a mid-range constant (soft-max/soft-min of c
    via exp(+-c/6) sums + float-bit ln).  Cauchy-Schwarz bounds the exponent
    by (c_k - G)/2, safely inside bf16/f32 range for G mid-range.
  * No explicit transpose or symmetrization anywhere: the AV matmul uses the
    q-major P tiles directly as lhsT.  Since S is symmetric, tile (j,jj) of P
    equals exp(S[q',k] - (c_k+G)/2) for q' in block jj, k in block j -- so
    with scaled values Vt[k] = t_k x[k] (t_k = exp((c_k-G)/2)) the per-k
    factors cancel: the accumulated result is num'[q'] = t_q' * num[q'].
    The stray t_q' folds into the existing per-row normalization scalar
    R = 1/(den_q * t_q) -- the elementwise P->E multiply of the symmetric
    formulation disappears entirely.
  * exp runs on ACT (bias per-partition, accum_out = row-sum denominators
    for free) for most j-steps; a subset of steps instead computes P on the
    gpsimd engine with a two-op Schraudolph bit-trick
        w = max(S + bias_q, -88);  P_bits(int16) = trunc(A16*w + B16)
    bitcast to bf16 (A16 = 128/ln2).  The clamp keeps the int16 conversion
    out of the NaN band; softmax normalization cancels the ~3% per-entry
    error.  Denominators for those steps come from a DVE row-reduce of P.
    This splits the exp workload across ACT/Pool/DVE so the tensor engine
    (QK^T + AV at their streaming rooflines) becomes the bottleneck.
  * QK^T and AV run in bf16 (f32 PSUM accumulation); rsqrt for LayerNorm is
    fast-inverse-sqrt + 2 Newton steps so ACT stays on one table set.
  * The two batches are software-pipelined: batch 1's main loop overlaps
    batch 0's output stage, and each engine's issue order is time-monotone.
"""

import sys

import numpy as np

sys.path.insert(0, "/opt/trn_rl_repo")

B, T, D = 16, 2048, 128
N_CORES = 8
NB = B // N_CORES          # batches per core
NT = T // 128              # 128-row tiles per batch
EPS = 1e-5

# per-j engines for the two 1024-wide exp halves: 'A' = ACT LUT exp,
# 'P' = gpsimd two-op Schraudolph.  Denominators for 'P' halves come from a
# DVE row-reduce of the bf16 P half-tile (full-tile AP -> 2x bf16 mode).
MODE = {j: ("A", "A") for j in range(16)}
for j in (2, 5, 8, 11, 14):
    MODE[j] = ("P", "P")
MODE[7] = ("A", "P")
# denominator engine for 'P' halves: 'V' = DVE tensor_reduce (1127ns),
# 'P' = gpsimd accum-sum into scratch (853ns)
DEN_ENG = {(j, h): "V" for j in range(16) for h in range(2)}
DEN_ENG[(2, 0)] = "P"
DEN_ENG[(14, 1)] = "P"
A16 = 128.0 / 0.6931471805599453        # bf16 Schraudolph scale
B16 = 16251.0                           # 127*128 - minimax shift + trunc comp
LN2 = 0.6931471805599453

_CACHE = {}


def _build():
    from contextlib import ExitStack

    import concourse.bacc as bacc
    import concourse.bass as bass  # noqa: F401
    import concourse.tile as tile
    from concourse import mybir

    f32 = mybir.dt.float32
    bf = mybir.dt.bfloat16
    i16 = mybir.dt.int16
    AF = mybir.ActivationFunctionType
    ALU = mybir.AluOpType
    AX = mybir.AxisListType

    nc = bacc.Bacc()

    x_d = nc.dram_tensor("x", [NB, T, D], f32, kind="ExternalInput")
    xT_d = nc.dram_tensor("xT", [NB, D, T], bf, kind="ExternalInput")
    g_d = nc.dram_tensor("gamma", [D], f32, kind="ExternalInput")
    b_d = nc.dram_tensor("beta", [D], f32, kind="ExternalInput")
    o_d = nc.dram_tensor("out", [NB, T, D], f32, kind="ExternalOutput")

    ctx = ExitStack()
    with tile.TileContext(nc) as tc, ctx:
        big = ctx.enter_context(tc.tile_pool(name="big", bufs=2))
        epool = ctx.enter_context(tc.tile_pool(name="epool", bufs=8))
        stats = ctx.enter_context(tc.tile_pool(name="stats", bufs=2))
        consts = ctx.enter_context(tc.tile_pool(name="consts", bufs=1))
        spool = ctx.enter_context(tc.tile_pool(name="spool", bufs=2, space="PSUM"))
        npool = ctx.enter_context(tc.tile_pool(name="npool", bufs=1, space="PSUM"))

        zero_t = consts.tile([128, 1], f32, tag="zero", name="zero")
        nc.vector.memset(zero_t, 0.0)
        ones_c = consts.tile([128, 1], f32, tag="ones_c", name="ones_c")
        nc.vector.memset(ones_c, 1.0)
        ones_r = consts.tile([1, 128], f32, tag="ones_r", name="ones_r")
        nc.vector.memset(ones_r, 1.0)

        def emit_loads(b, st):
            st["xT"] = big.tile([128, T], bf, tag="xT", name="xT")
            st["x"] = big.tile([128, NT, D], f32, tag="x", name="x")
            xv = x_d[b].rearrange("(t p) d -> p t d", p=128)
            for sx in range(4):
                nc.sync.dma_start(
                    out=st["x"][:, sx * 4 : (sx + 1) * 4, :],
                    in_=xv[:, sx * 4 : (sx + 1) * 4, :],
                )

        def emit_loads_xT(b, st):
            for sx in range(2):
                nc.sync.dma_start(
                    out=st["xT"][:, sx * 1024 : (sx + 1) * 1024],
                    in_=xT_d[b, :, sx * 1024 : (sx + 1) * 1024],
                )

        def emit_stats(b, st):
            x_sb = st["x"]
            C = stats.tile([128, NT], f32, tag="C", name="C")
            sqb = big.tile([128, NT, D], f32, tag="sqb", name="sqb")
            for t in range(NT):
                nc.gpsimd.scalar_tensor_tensor(
                    out=sqb[:, t, :],
                    in0=x_sb[:, t, :],
                    scalar=1.0,
                    in1=x_sb[:, t, :],
                    op0=ALU.mult,
                    op1=ALU.mult,
                    accum_out=C[:, t : t + 1],
                )
            # Soft bounds on the range of c without cross-partition reductions:
            #   cbar = 6 ln(sum exp(c/6))  in [max c, max c + 6 ln 2048]
            #   mbar = -6 ln(sum exp(-c/6)) in [min c - 6 ln 2048, min c]
            # then G = (cbar+mbar)/2 is a mid-range shift.  The ln's use
            # Schraudolph float-bits; cross-partition sum and the broadcast
            # back are K=1/M=2 matmuls on the PE.
            EC = stats.tile([128, NT], f32, tag="EC", name="EC")
            nc.scalar.activation(out=EC, in_=C, func=AF.Exp, bias=zero_t, scale=1.0 / 6.0)
            ECm = stats.tile([128, NT], f32, tag="ECm", name="ECm")
            nc.scalar.activation(out=ECm, in_=C, func=AF.Exp, bias=zero_t, scale=-1.0 / 6.0)
            ec2 = stats.tile([128, 2], f32, tag="ec2", name="ec2")
            nc.vector.tensor_reduce(out=ec2[:, 0:1], in_=EC, axis=AX.X, op=ALU.add)
            nc.vector.tensor_reduce(out=ec2[:, 1:2], in_=ECm, axis=AX.X, op=ALU.add)
            s1a = spool.tile([1, 1], f32, tag="S", name="s1a")
            nc.tensor.matmul(out=s1a, lhsT=ec2[:, 0:1], rhs=ones_c, start=True, stop=True)
            s1b = spool.tile([1, 1], f32, tag="S", name="s1b")
            nc.tensor.matmul(out=s1b, lhsT=ec2[:, 1:2], rhs=ones_c, start=True, stop=True)
            LL2 = stats.tile([1, 2], f32, tag="LL2", name="LL2")
            nc.vector.tensor_copy(out=LL2[0:1, 0:1], in_=s1a.bitcast(mybir.dt.int32))
            nc.vector.tensor_copy(out=LL2[0:1, 1:2], in_=s1b.bitcast(mybir.dt.int32))
            s2 = spool.tile([128, 2], f32, tag="S", name="s2")
            nc.tensor.matmul(out=s2, lhsT=ones_r, rhs=LL2, start=True, stop=True)
            # G = (cbar+mbar)/2 = 3*LN2*(bits_plus - bits_minus)/2^23  [128,1]
            s2s = stats.tile([128, 2], f32, tag="s2s", name="s2s")
            nc.vector.tensor_copy(out=s2s, in_=s2)
            Gd = stats.tile([128, 1], f32, tag="Gd", name="Gd")
            nc.vector.tensor_tensor(
                out=Gd, in0=s2s[:, 0:1], in1=s2s[:, 1:2], op=ALU.subtract
            )
            Gc = stats.tile([128, 1], f32, tag="Gc", name="Gc")
            nc.vector.tensor_scalar_mul(out=Gc, in0=Gd, scalar1=3.0 * LN2 / 8388608.0)
            Gh_neg = stats.tile([128, 1], f32, tag="Ghn", name="Ghn")
            nc.vector.tensor_scalar_mul(out=Gh_neg, in0=Gc, scalar1=-0.5)
            # bias_all[:, t] = -(c + G)/2
            bias_all = stats.tile([128, NT], f32, tag="bias", name="bias")
            nc.vector.tensor_scalar(
                out=bias_all,
                in0=C,
                scalar1=Gc,
                scalar2=-0.5,
                op0=ALU.add,
                op1=ALU.mult,
            )
            st["bias"] = bias_all
            # t_k = exp((c_k - G)/2); scaled values Vt = t * x (bf16)
            Tall = stats.tile([128, NT], f32, tag="Tall", name="Tall")
            nc.scalar.activation(out=Tall, in_=C, func=AF.Exp, bias=Gh_neg, scale=0.5)
            st["Tall"] = Tall
            Vt = big.tile([128, NT, D], bf, tag="Vt", name="Vt")
            for t in range(NT):
                nc.gpsimd.tensor_scalar_mul(
                    out=Vt[:, t, :], in0=x_sb[:, t, :], scalar1=Tall[:, t : t + 1]
                )
            st["Vt"] = Vt
            st["Den"] = stats.tile([128, NT, 2], f32, tag="Den", name="Den")
            nc.vector.memset(st["Den"], 0.0)

        def emit_qk_exp(b, st, j):
            # QK^T for row-block j + exp into two bf16 half-tiles (full-tile
            # APs so DVE reduces get the 2x bf16 mode).  The AV matmuls for
            # block j are emitted one step later (emit_av) so PE work overlaps
            # the exp of the next block instead of gating it.
            if j == 0:
                st["num"] = npool.tile([128, T], f32, tag="num", name="num")
                st["E"] = {}
            Eh = [
                epool.tile([128, 1024], bf, tag="E0", name="E0"),
                epool.tile([128, 1024], bf, tag="E1", name="E1"),
            ]
            st["E"][j] = Eh
            xT_sb = st["xT"]
            for h in range(2):
                S = spool.tile([128, 1024], f32, tag="S", name="S")
                for q in range(2):
                    n0 = h * 1024 + q * 512
                    nc.tensor.matmul(
                        out=S[:, q * 512 : (q + 1) * 512],
                        lhsT=xT_sb[:, j * 128 : (j + 1) * 128],
                        rhs=xT_sb[:, n0 : n0 + 512],
                        start=True,
                        stop=True,
                    )
                if MODE[j][h] == "P":
                    # Pool 2-op Schraudolph: w = max(S+bias,-88);
                    # bits = trunc(A16*w + B16) -> int16 view of bf16 tile
                    tmp = big.tile([128, 1024], f32, tag="stmp", name="stmp")
                    nc.gpsimd.tensor_scalar(
                        out=tmp,
                        in0=S,
                        scalar1=st["bias"][:, j : j + 1],
                        scalar2=-88.0,
                        op0=ALU.add,
                        op1=ALU.max,
                    )
                    nc.gpsimd.tensor_scalar(
                        out=Eh[h].bitcast(i16),
                        in0=tmp,
                        scalar1=A16,
                        scalar2=B16,
                        op0=ALU.mult,
                        op1=ALU.add,
                    )
                    if DEN_ENG[(j, h)] == "V":
                        nc.vector.tensor_reduce(
                            out=st["Den"][:, j, h : h + 1], in_=Eh[h], axis=AX.X,
                            op=ALU.add,
                        )
                    else:
                        dscr = big.tile([128, 1024], bf, tag="dscr", name="dscr", bufs=1)
                        nc.gpsimd.tensor_scalar(
                            out=dscr,
                            in0=Eh[h],
                            scalar1=1.0,
                            scalar2=0.0,
                            op0=ALU.mult,
                            op1=ALU.add,
                            accum_out=st["Den"][:, j, h : h + 1],
                        )
                else:
                    nc.scalar.activation(
                        out=Eh[h],
                        in_=S,
                        func=AF.Exp,
                        bias=st["bias"][:, j : j + 1],
                        scale=1.0,
                        accum_out=st["Den"][:, j, h : h + 1],
                    )

        def emit_av(b, st, j):
            Eh = st["E"].pop(j)
            for jj in range(NT):
                # 4 output slices share a 2KB PSUM bank = one zero region:
                # only the bank's first MM sets start, only its last sets stop
                nc.tensor.matmul(
                    out=st["num"][:, jj * 128 : (jj + 1) * 128],
                    lhsT=Eh[jj // 8][:, (jj % 8) * 128 : (jj % 8 + 1) * 128],
                    rhs=st["Vt"][:, j, :],
                    start=(j == 0 and jj % 4 == 0),
                    stop=(j == NT - 1 and jj % 4 == 3),
                )

        def emit_den(b, st):
            den = stats.tile([128, NT], f32, tag="den", name="den")
            nc.vector.tensor_reduce(out=den, in_=st["Den"], axis=AX.X, op=ALU.add)
            denT = stats.tile([128, NT], f32, tag="denT", name="denT")
            nc.vector.tensor_tensor(out=denT, in0=den, in1=st["Tall"], op=ALU.mult)
            R = stats.tile([128, NT], f32, tag="R", name="R")
            nc.vector.reciprocal(out=R, in_=denT)
            st["R"] = R

        def emit_drain(b, st, copy_psum=True, half=None):
            # drain AV results out of PSUM so the next batch can reuse it
            # (skipped for the last batch -- nothing needs the banks).
            # Staggered: half 0 at the phase boundary, half 1 a few iterations
            # later, so the copies don't starve DVE mid-phase.
            if copy_psum:
                if half != 1:
                    st["numS"] = big.tile([128, T], f32, tag="numS", name="numS")
                for h in ([0, 1] if half is None else [half]):
                    nc.vector.tensor_copy(
                        out=st["numS"][:, h * 1024 : (h + 1) * 1024],
                        in_=st["num"][:, h * 1024 : (h + 1) * 1024],
                    )
            else:
                st["numS"] = st["num"]
            if half != 1:
                st["Y"] = big.tile([128, NT, D], f32, tag="Y", name="Y")
                st["MV"] = stats.tile([128, NT, 2], f32, tag="MV", name="MV")
                st["Yout"] = big.tile([128, NT, D], f32, tag="Yout", name="Yout")

        def emit_outA(b, st, jj, act_stats=False):
            # y = num'/den' + x.  LN stats either via DVE bn_stats (b0: DVE has
            # slack mid-phase) or via accum_out (b1 tail: split DVE/Pool).
            eng = nc.gpsimd if (act_stats and jj % 2 == 1) else nc.vector
            eng.scalar_tensor_tensor(
                out=st["Y"][:, jj, :],
                in0=st["numS"][:, jj * 128 : (jj + 1) * 128],
                scalar=st["R"][:, jj : jj + 1],
                in1=st["x"][:, jj, :],
                op0=ALU.mult,
                op1=ALU.add,
                accum_out=st["Sy"][:, jj : jj + 1] if act_stats else None,
            )
            if act_stats:
                eng.scalar_tensor_tensor(
                    out=st["ysqb"][:, jj, :],
                    in0=st["Y"][:, jj, :],
                    scalar=1.0,
                    in1=st["Y"][:, jj, :],
                    op0=ALU.mult,
                    op1=ALU.mult,
                    accum_out=st["Sy2"][:, jj : jj + 1],
                )
            else:
                bns = stats.tile([128, 6], f32, tag="bns", name="bns")
                nc.vector.bn_stats(out=bns, in_=st["Y"][:, jj, :])
                nc.vector.bn_aggr(out=st["MV"][:, jj, :], in_=bns)

        def emit_lnr(b, st, act_stats=False, lo=0, hi=NT):
            cs = slice(lo, hi)
            if act_stats:
                if "mu" not in st:
                    st["mu"] = stats.tile([128, NT], f32, tag="mu", name="mu")
                    st["vart"] = stats.tile([128, NT], f32, tag="vart", name="vart")
                    st["rstd"] = stats.tile([128, NT], f32, tag="rstd", name="rstd")
                # mu = Sy/128, var = Sy2/128 - mu^2
                nc.vector.tensor_scalar_mul(
                    out=st["mu"][:, cs], in0=st["Sy"][:, cs], scalar1=1.0 / D
                )
                musq = stats.tile([128, NT], f32, tag="musq", name="musq")
                nc.vector.scalar_tensor_tensor(
                    out=musq[:, cs],
                    in0=st["mu"][:, cs],
                    scalar=1.0,
                    in1=st["mu"][:, cs],
                    op0=ALU.mult,
                    op1=ALU.mult,
                )
                nc.vector.scalar_tensor_tensor(
                    out=st["vart"][:, cs],
                    in0=st["Sy2"][:, cs],
                    scalar=1.0 / D,
                    in1=musq[:, cs],
                    op0=ALU.mult,
                    op1=ALU.subtract,
                )
                var_in = st["vart"][:, cs]
            else:
                if "rstd" not in st:
                    st["rstd"] = stats.tile([128, NT], f32, tag="rstd", name="rstd")
                var_in = st["MV"][:, cs, 1]
            # rstd = 1/sqrt(var+eps) via the fast-inverse-sqrt bit trick plus
            # two Newton steps (~4e-6 rel err) -- keeps the ACT engine on the
            # exp table set for the whole kernel (table swaps cost 1.3us each)
            ve = stats.tile([128, NT], f32, tag="ve", name="ve")
            nc.vector.tensor_scalar_add(out=ve[:, cs], in0=var_in, scalar1=EPS)
            wf = stats.tile([128, NT], f32, tag="wf", name="wf")
            nc.vector.tensor_copy(out=wf[:, cs], in_=ve[:, cs].bitcast(mybir.dt.int32))
            nc.vector.tensor_scalar(
                out=wf[:, cs], in0=wf[:, cs],
                scalar1=-0.5, scalar2=1597463007.0,
                op0=ALU.mult, op1=ALU.add,
            )
            wi = stats.tile([128, NT], mybir.dt.int32, tag="wi", name="wi")
            nc.vector.tensor_copy(out=wi[:, cs], in_=wf[:, cs])
            y = stats.tile([128, NT], f32, tag="y0", name="y0")
            nc.vector.tensor_copy(out=y[:, cs], in_=wi[:, cs].bitcast(f32))
            t1 = stats.tile([128, NT], f32, tag="t1", name="t1")
            for _ in range(2):
                nc.vector.tensor_mul(out=t1[:, cs], in0=ve[:, cs], in1=y[:, cs])
                nc.vector.tensor_mul(out=t1[:, cs], in0=t1[:, cs], in1=y[:, cs])
                nc.vector.tensor_scalar(
                    out=t1[:, cs], in0=t1[:, cs],
                    scalar1=-0.5, scalar2=1.5, op0=ALU.mult, op1=ALU.add,
                )
                nc.vector.tensor_mul(out=y[:, cs], in0=y[:, cs], in1=t1[:, cs])
            nc.vector.tensor_copy(out=st["rstd"][:, cs], in_=y[:, cs])

        def emit_outB(b, st, jj, act_stats=False):
            mu_s = st["mu"][:, jj : jj + 1] if act_stats else st["MV"][:, jj, 0:1]
            z = stats.tile([128, D], f32, tag="z", name="z")
            zeng = nc.gpsimd if (act_stats and jj % 2 == 1) else nc.vector
            zeng.tensor_scalar(
                out=z,
                in0=st["Y"][:, jj, :],
                scalar1=mu_s,
                scalar2=st["rstd"][:, jj : jj + 1],
                op0=ALU.subtract,
                op1=ALU.mult,
            )
            z2 = stats.tile([128, D], f32, tag="z2", name="z2")
            nc.gpsimd.tensor_mul(out=z2, in0=z, in1=gb)
            nc.gpsimd.tensor_add(out=st["Yout"][:, jj, :], in0=z2, in1=bb)

        def emit_outdma(b, st, half=None, quarter=None):
            ov = o_d[b].rearrange("(t p) d -> p t d", p=128)
            if quarter is not None:
                q4 = slice(quarter * 4, (quarter + 1) * 4)
                nc.sync.dma_start(out=ov[:, q4, :], in_=st["Yout"][:, q4, :])
            elif half is None:
                nc.sync.dma_start(out=ov, in_=st["Yout"])
            else:
                h8 = slice(half * 8, (half + 1) * 8)
                nc.sync.dma_start(out=ov[:, h8, :], in_=st["Yout"][:, h8, :])

        # ---- software-pipelined schedule over the two batches ---------------
        A, Bst = {}, {}
        emit_loads(0, A)
        emit_stats(0, A)
        emit_loads_xT(0, A)
        emit_loads(1, Bst)
        emit_loads_xT(1, Bst)
        gb = consts.tile([128, D], f32, tag="gb", name="gb")
        bb = consts.tile([128, D], f32, tag="bb", name="bb")
        for j in range(NT):
            emit_qk_exp(0, A, j)
            if j > 0:
                emit_av(0, A, j - 1)
            if j == 3:
                emit_stats(1, Bst)
            if j == 5:
                nc.sync.dma_start(out=gb, in_=g_d[:].partition_broadcast(128))
                nc.sync.dma_start(out=bb, in_=b_d[:].partition_broadcast(128))
        emit_av(0, A, NT - 1)
        emit_den(0, A)
        emit_drain(0, A, half=0)
        # phase 1: batch 1's main loop with batch 0's whole output stage
        # threaded through it (outA x2 in early iters, lnr at 8, outB x2 late)
        for j in range(NT):
            emit_qk_exp(1, Bst, j)
            if j > 0:
                emit_av(1, Bst, j - 1)
            if j == 2:
                emit_drain(0, A, half=1)
            if j < 8:
                emit_outA(0, A, 2 * j)
                emit_outA(0, A, 2 * j + 1)
            else:
                if j == 8:
                    emit_lnr(0, A)
                emit_outB(0, A, 2 * (j - 8))
                emit_outB(0, A, 2 * (j - 8) + 1)
                if j == 12:
                    emit_outdma(0, A, half=0)
        emit_av(1, Bst, NT - 1)
        emit_outdma(0, A, half=1)
        emit_den(1, Bst)
        emit_drain(1, Bst, copy_psum=False)
        Bst["Sy"] = stats.tile([128, NT], f32, tag="Sy", name="Sy")
        Bst["Sy2"] = stats.tile([128, NT], f32, tag="Sy2", name="Sy2")
        Bst["ysqb"] = big.tile([128, NT, D], f32, tag="ysqb", name="ysqb", bufs=1)
        # half-split tail: LN stats for tiles 0-7 finish while 8-15 are still
        # accumulating, so normalize+store of the first half overlaps the rest
        for jj in range(8):
            emit_outA(1, Bst, jj, act_stats=True)
        emit_lnr(1, Bst, act_stats=True, lo=0, hi=8)
        for jj in range(8):
            emit_outA(1, Bst, jj + 8, act_stats=True)
            emit_outB(1, Bst, jj, act_stats=True)
        emit_outdma(1, Bst, half=0)
        emit_lnr(1, Bst, act_stats=True, lo=8, hi=NT)
        for jj in range(8, NT):
            emit_outB(1, Bst, jj, act_stats=True)
            if jj == 11:
                emit_outdma(1, Bst, quarter=2)
        emit_outdma(1, Bst, quarter=3)

    nc.finalize()
    return nc


def _get_nc():
    if "nc" not in _CACHE:
        _CACHE["nc"] = _build()
    return _CACHE["nc"]


def _run(x, gamma, beta, trace=False):
    import ml_dtypes

    from concourse.bass_utils import run_bass_kernel_spmd

    x = np.ascontiguousarray(np.asarray(x, dtype=np.float32))
    gamma = np.ascontiguousarray(np.asarray(gamma, dtype=np.float32))
    beta = np.ascontiguousarray(np.asarray(beta, dtype=np.float32))

    xs = x.reshape(N_CORES, NB, T, D)
    xTs = np.ascontiguousarray(xs.transpose(0, 1, 3, 2)).astype(ml_dtypes.bfloat16)

    in_maps = [
        {
            "x": np.ascontiguousarray(xs[c]),
            "xT": xTs[c],
            "gamma": gamma,
            "beta": beta,
        }
        for c in range(N_CORES)
    ]
    res = run_bass_kernel_spmd(
        _get_nc(), in_maps, core_ids=list(range(N_CORES)), trace=trace
    )
    out = np.stack([res.results[c]["out"] for c in range(N_CORES)], axis=0)
    return out.reshape(B, T, D), res


def kernel(x, gamma, beta):
    out, _ = _run(x, gamma, beta, trace=False)
    return out
